# revision 32
# baseline (speedup 1.0000x reference)
import numpy as np
import ml_dtypes

import concourse.bass as bass
import concourse.mybir as mybir
import concourse.tile as tile
from concourse import bacc
from concourse.bass_utils import run_bass_kernel_spmd

B, T, HWs, D, N, C = 16, 64, 7, 768, 12, 64
HW2 = HWs * HWs          # 49
NCORES = 8
BLOC = B // NCORES       # 2 batches per core
BT = BLOC * T            # 128 (b,t) rows per core
ROWS = BT * HW2          # 6272
KT = D // 128            # 6 k-chunks for the qkv projection
NCOL = 3 * D             # 2304
EPS = 1e-5
SCALE = C ** -0.5        # folded into k's layernorm gamma/beta
FCH = [(i * 512, 512) for i in range(12)] + [(6144, 128)]  # 6272 free chunks
REL = 2 * T - 1          # 127

bf16 = mybir.dt.bfloat16
f32 = mybir.dt.float32
AF = mybir.ActivationFunctionType

_cache = {}


def _build_nc():
    nc = bacc.Bacc(None, target_bir_lowering=False, debug=False)
    xT = nc.declare_dram_parameter("xT", [KT, 128, ROWS], bf16, isOutput=False)
    Wq = nc.declare_dram_parameter("Wq", [KT, 128, NCOL], bf16, isOutput=False)
    Wpd = nc.declare_dram_parameter("Wpd", [3, 128, HW2 * 128], bf16, isOutput=False)
    bp = nc.declare_dram_parameter("bp", [1, 3 * 128], bf16, isOutput=False)
    lng = nc.declare_dram_parameter("lng", [3, 128, C], f32, isOutput=False)
    lnb = nc.declare_dram_parameter("lnb", [3, 128, C], f32, isOutput=False)
    relT = nc.declare_dram_parameter("relT", [128, REL], bf16, isOutput=False)
    idbf = nc.declare_dram_parameter("idbf", [128, 128], bf16, isOutput=False)
    idf = nc.declare_dram_parameter("idf", [128, 128], f32, isOutput=False)
    Wpr = nc.declare_dram_parameter("Wpr", [KT, 128, D], bf16, isOutput=False)
    bpr = nc.declare_dram_parameter("bpr", [1, D], bf16, isOutput=False)
    onesv = nc.declare_dram_parameter("onesv", [1, BT], bf16, isOutput=False)
    out = nc.declare_dram_parameter("out", [BT, D], f32, isOutput=True)

    with tile.TileContext(nc) as tc:
        with (
            tc.tile_pool(name="cp", bufs=1) as cp,
            tc.tile_pool(name="qp", bufs=2) as qp,
            tc.tile_pool(name="sp", bufs=2) as sp,
            tc.tile_pool(name="vp", bufs=5) as vp,
            tc.tile_pool(name="ap", bufs=7) as apl,
            tc.tile_pool(name="ab", bufs=9) as abp,
            tc.tile_pool(name="dp", bufs=2, space="DRAM") as dp,
            tc.tile_pool(name="ps1", bufs=3, space="PSUM") as ps1,
            tc.tile_pool(name="psP", bufs=1, space="PSUM") as psP,
            tc.tile_pool(name="psX", bufs=4, space="PSUM") as psX,
        ):
            # ---- resident constants; spread loads over DMA queues and
            # order them so (xT[k], Wq[k]) pairs arrive in k order ----
            engs = [nc.sync, nc.scalar, nc.gpsimd]
            xt, wt, wpt = [], [], []
            for k in range(KT):
                w = cp.tile([128, NCOL], bf16, tag=f"w{k}")
                wt.append(w)
                xk = cp.tile([128, ROWS], bf16, tag=f"x{k}")
                xt.append(xk)
            ei = 0
            for k in range(KT):
                engs[ei % 3].dma_start(wt[k][:], Wq[k])
                ei += 1
                engs[ei % 3].dma_start(xt[k][:, 0:1568], xT[k][:, 0:1568])
                ei += 1
            for q in range(1, 4):
                for k in range(KT):
                    engs[ei % 3].dma_start(
                        xt[k][:, q * 1568:(q + 1) * 1568],
                        xT[k][:, q * 1568:(q + 1) * 1568])
                    ei += 1
            idb = cp.tile([128, 128], bf16, tag="idbf")
            nc.sync.dma_start(idb[:], idbf[:])
            idf_ = cp.tile([128, 128], f32, tag="idf")
            nc.scalar.dma_start(idf_[:], idf[:])
            bpt = cp.tile([1, 3 * 128], bf16, tag="bp")
            nc.gpsimd.dma_start(bpt[:], bp[:])
            gt = cp.tile([128, 3 * C], f32, tag="lng")
            bt_ = cp.tile([128, 3 * C], f32, tag="lnb")
            for j in range(3):
                engs[j % 3].dma_start(gt[:, j * C:(j + 1) * C], lng[j])
                engs[(j + 1) % 3].dma_start(bt_[:, j * C:(j + 1) * C], lnb[j])
            rlt = cp.tile([128, REL], bf16, tag="relT")
            nc.gpsimd.dma_start(rlt[:], relT[:])
            ot_ = cp.tile([1, BT], bf16, tag="ones")
            nc.sync.dma_start(ot_[:], onesv[:])
            for j in range(3):
                t_ = cp.tile([128, HW2 * 128], bf16, tag=f"wp{j}")
                engs[j % 3].dma_start(t_[:], Wpd[j])
                wpt.append(t_)
            wprt = []
            for k in range(KT):
                t_ = cp.tile([128, D], bf16, tag=f"wpr{k}")
                engs[k % 3].dma_start(t_[:], Wpr[k])
                wprt.append(t_)
            bprt = cp.tile([1, D], bf16, tag="bpr")
            nc.scalar.dma_start(bprt[:], bpr[:])

            attc = [None] * 6
            osb = cp.tile([BT, D], f32, tag="osb")
            st = {}                      # per-nh pipeline state
            pend = []                    # deferred layernorm tails
            pftiles = []                 # proj psum accumulators

            def flush_ln_tails(keep=0):
                # apply rs*g + b one block later so DVE never waits on the
                # ACT sqrt (table-load) in its in-order stream
                while len(pend) > keep:
                    j, xc, sd, ln = pend.pop(0)
                    rs = sp.tile([128, 1], f32, tag="rs")
                    nc.vector.reciprocal(rs[:], sd[:])
                    nc.vector.tensor_scalar_mul(xc[:], xc[:], rs[:])
                    nc.vector.tensor_mul(xc[:], xc[:], gt[:, j * C:(j + 1) * C])
                    nc.vector.tensor_add(ln, xc[:], bt_[:, j * C:(j + 1) * C])

            def emit_s1_pool(j, nh, qkv_ln, flush=True):
                    p = j * 6 + nh
                    # ---- stage 1: qkv projection, column tile p ----
                    qkvT = qp.tile([128, ROWS], bf16, tag="qkvT")
                    for ci in range(0, len(FCH), 2):
                        pair = FCH[ci:ci + 2]
                        pts = []
                        for _ in pair:
                            pt = ps1.tile([128, 512], f32, tag="s1")
                            pts.append(pt)
                        for k in range(KT):
                            for pt, (foff, fsz) in zip(pts, pair):
                                nc.tensor.matmul(
                                    pt[:, :fsz],
                                    wt[k][:, p * 128:(p + 1) * 128],
                                    xt[k][:, foff:foff + fsz],
                                    start=(k == 0), stop=(k == KT - 1))
                        for pt, (foff, fsz) in zip(pts, pair):
                            nc.vector.tensor_copy(qkvT[:, foff:foff + fsz],
                                                  pt[:, :fsz])
                    # ---- stage 2: pooling conv, block-diagonal weights do
                    # both heads of the tile in one k=128 matmul per tap ----
                    qv = qkvT[:].rearrange("p (bt hw) -> p hw bt", hw=HW2)
                    pp = psP.tile([128, BT], f32, tag="pool")
                    for hw in range(HW2):
                        nc.tensor.matmul(
                            pp[:],
                            wpt[j][:, hw * 128:(hw + 1) * 128],
                            qv[:, hw:hw + 1, :],
                            start=(hw == 0), stop=False)
                    nc.tensor.matmul(
                        pp[:], bpt[0:1, j * 128:(j + 1) * 128], ot_[0:1, :],
                        start=False, stop=True)
                    pTs = sp.tile([128, BT], f32, tag="poolT")
                    nc.scalar.copy(pTs[:], pp[:])
                    pn = psX.tile([128, 128], f32, tag="x")
                    nc.tensor.transpose(pn[:], pTs[:], idf_[:, 0:128])
                    # ---- layernorm over c per head (free-dim slices) ----
                    lntile = vp.tile([128, 128], bf16, tag=f"ln{j}")
                    for u in range(2):
                        q0 = u * 64
                        pnu = pn[:, q0:q0 + 64]
                        m_ = sp.tile([128, 1], f32, tag="m")
                        nc.vector.reduce_sum(m_[:], pnu, axis=mybir.AxisListType.X)
                        nc.vector.tensor_scalar_mul(m_[:], m_[:], 1.0 / C)
                        xc = vp.tile([128, C], f32, tag="xc")
                        nc.vector.tensor_scalar_sub(xc[:], pnu, m_[:])
                        sq = sp.tile([128, C], f32, tag="sq")
                        nc.vector.tensor_mul(sq[:], xc[:], xc[:])
                        v_ = sp.tile([128, 1], f32, tag="v")
                        nc.vector.reduce_sum(v_[:], sq[:], axis=mybir.AxisListType.X)
                        nc.vector.tensor_scalar(v_[:], v_[:], 1.0 / C, EPS,
                                                op0=mybir.AluOpType.mult,
                                                op1=mybir.AluOpType.add)
                        sd = vp.tile([128, 1], f32, tag="sd")
                        nc.scalar.sqrt(sd[:], v_[:])
                        pend.append((j, xc, sd, lntile[:, q0:q0 + 64]))
                    qkv_ln[j] = lntile
                    if flush:
                        flush_ln_tails(keep=2)

            def emit_attnA(nh, qkv_ln):
                # qT/kT transposes, rel-bias E + shear round-trip, S matmuls
                qT = vp.tile([128, BT], bf16, tag="qT")
                kT = vp.tile([128, BT], bf16, tag="kT")
                for u in range(2):
                    for srct, dstt in ((qkv_ln[0], qT), (qkv_ln[1], kT)):
                        tp = psX.tile([128, BT], bf16, tag="x")
                        nc.tensor.transpose(tp[0:64, :],
                                            srct[:, u * 64:u * 64 + 64],
                                            idb[:, 0:BT])
                        nc.scalar.copy(dstt[u * 64:u * 64 + 64, :], tp[0:64, :])
                Esb = sp.tile([64, 4 * REL], f32, tag="Esb")
                for u in range(2):
                    for b in range(2):
                        g = u * 2 + b
                        ep = psX.tile([64, REL], f32, tag="x")
                        nc.tensor.matmul(
                            ep[:],
                            qT[u * 64:u * 64 + 64, b * 64:(b + 1) * 64],
                            rlt[u * 64:u * 64 + 64, :],
                            start=True, stop=True)
                        nc.scalar.copy(Esb[:, g * REL:(g + 1) * REL], ep[:])
                ed = dp.tile([64, 4 * REL], f32, tag="Ed")
                nc.sync.dma_start(ed[:], Esb[:])
                rel = sp.tile([64, 4 * T], f32, tag="rel")
                for g in range(4):
                    shear = bass.AP(
                        tensor=ed[:].tensor,
                        offset=ed[:].offset + g * REL + T - 1,
                        ap=[[4 * REL + 1, 64], [-1, T]])
                    nc.sync.dma_start(rel[:, g * T:(g + 1) * T], shear)
                Ssb = sp.tile([64, 4 * T], f32, tag="Ssb")
                for u in range(2):
                    for b in range(2):
                        g = u * 2 + b
                        sps = psX.tile([64, T], f32, tag="x")
                        nc.tensor.matmul(
                            sps[:],
                            qT[u * 64:u * 64 + 64, b * 64:(b + 1) * 64],
                            kT[u * 64:u * 64 + 64, b * 64:(b + 1) * 64],
                            start=True, stop=True)
                        nc.scalar.copy(Ssb[:, g * T:(g + 1) * T], sps[:])
                return qT, kT, rel, Ssb

            def emit_attnB1(nh, qkv_ln, qT, kT, rel, Ssb):
                exs, dens = [], []
                for g in range(4):
                    ain = sp.tile([64, T], f32, tag="ain")
                    nc.vector.tensor_add(ain[:], Ssb[:, g * T:(g + 1) * T],
                                         rel[:, g * T:(g + 1) * T])
                    den = abp.tile([64, 1], f32, tag="den")
                    ex = abp.tile([64, T], f32, tag="ex")
                    nc.scalar.activation(ex[:], ain[:], AF.Exp,
                                         accum_out=den[:])
                    exs.append(ex)
                    dens.append(den)
                return exs, dens

            def emit_attnB2(nh, qkv_ln, qT, exs, dens):
                for u in range(2):
                    q0 = u * 64
                    for b in range(2):
                        g = u * 2 + b
                        b0 = b * 64
                        rr = sp.tile([64, 1], f32, tag="rr")
                        nc.vector.reciprocal(rr[:], dens[g][:])
                        ab = sp.tile([64, T], bf16, tag="ab")
                        nc.vector.tensor_scalar_mul(ab[:], exs[g][:], rr[:])
                        atp = psX.tile([64, T], bf16, tag="x")
                        nc.tensor.transpose(atp[:], ab[:], idb[0:64, 0:T])
                        ats = sp.tile([128, T], bf16, tag="ats")
                        nc.scalar.copy(ats[b0:b0 + 64, :], atp[:])
                        oup = psX.tile([64, C], f32, tag="x")
                        nc.tensor.matmul(
                            oup[:],
                            ats[b0:b0 + 64, :],
                            qkv_ln[2][b0:b0 + 64, u * 64:u * 64 + 64],
                            start=True, stop=True)
                        ousb = sp.tile([64, C], bf16, tag="ousb")
                        nc.scalar.copy(ousb[:], oup[:])
                        otp = psX.tile([64, T], bf16, tag="x")
                        nc.tensor.transpose(otp[:], ousb[:], idb[0:64, 0:T])
                        if attc[nh] is None:
                            ac_t = apl.tile([128, BT], bf16, tag="attc")
                            attc[nh] = ac_t
                        nc.vector.tensor_add(
                            attc[nh][q0:q0 + 64, b0:b0 + 64], otp[:],
                            qT[q0:q0 + 64, b0:b0 + 64])

            for nh in range(6):          # head pair, attn pipelined one behind
                qkv_ln = [None, None, None]
                st[nh] = qkv_ln
                for j in range(3):
                    emit_s1_pool(j, nh, qkv_ln, flush=(j != 0))
                    if j == 0 and nh > 0:
                        st[nh - 1] = (st[nh - 1],
                                      *emit_attnA(nh - 1, st[nh - 1]))
                        flush_ln_tails(keep=2)
                        if nh > 1:
                            pln, pqT, pexs, pdens = st.pop(nh - 2)
                            emit_attnB2(nh - 2, pln, pqT, pexs, pdens)
                    if j == 1 and nh == 5:
                        # last gen: attnA only needs q/k -> emit before the
                        # v tile so its DMA round-trip hides under s1(2,5)
                        flush_ln_tails()
                        st[5] = (st[5], *emit_attnA(5, st[5]))
                    if j == 2 and nh > 0:
                        pln, pqT, pkT, prel, pSsb = st[nh - 1]
                        exs, dens = emit_attnB1(nh - 1, pln, pqT, pkT,
                                                prel, pSsb)
                        st[nh - 1] = (pln, pqT, exs, dens)
                        if nh == 5:
                            pln, pqT, pkT, prel, pSsb = st[5]
                            exs, dens = emit_attnB1(5, pln, pqT, pkT,
                                                    prel, pSsb)
                            st[5] = (pln, pqT, exs, dens)
            pln, pqT, pexs, pdens = st.pop(4)
            emit_attnB2(4, pln, pqT, pexs, pdens)
            for f in range(2):
                f0 = f * 384
                pfs = psX.tile([128, 384], f32, tag="x")
                for c_ in range(5):
                    nc.tensor.matmul(
                        pfs[:], attc[c_][:], wprt[c_][:, f0:f0 + 384],
                        start=(c_ == 0), stop=(c_ == 4))
                pfsb = sp.tile([128, 384], f32, tag="pfsb")
                pftiles.append(pfsb)
                nc.scalar.copy(pfsb[:], pfs[:])
            flush_ln_tails()
            pln, pqT, pexs, pdens = st.pop(5)
            emit_attnB2(5, pln, pqT, pexs, pdens)

            # ---- output projection: last chunk + bias, then merge ----
            for f in range(2):
                f0 = f * 384
                pf2 = psX.tile([128, 384], f32, tag="x")
                nc.tensor.matmul(pf2[:], attc[5][:], wprt[5][:, f0:f0 + 384],
                                 start=True, stop=False)
                nc.tensor.matmul(pf2[:], ot_[0:1, :], bprt[0:1, f0:f0 + 384],
                                 start=False, stop=True)
                nc.vector.tensor_add(osb[:, f0:f0 + 384], pf2[:], pftiles[f][:])
            nc.sync.dma_start(out[:], osb[:])
    nc.compile()
    return nc


def _prep_consts(W_qkv, Wpq, bpq, Wpk, bpk, Wpv, bpv,
                 g_q, be_q, g_k, be_k, g_v, be_v, rel_pos_t, W_proj, b_proj):
    f = np.float32
    con = {}
    con["Wq"] = np.ascontiguousarray(
        np.asarray(W_qkv, f).reshape(KT, 128, NCOL)).astype(ml_dtypes.bfloat16)
    wpd = []
    for Wp in (Wpq, Wpk, Wpv):
        W2 = np.asarray(Wp, f).transpose(2, 3, 1, 0).reshape(HW2, C, C)
        blk = np.zeros((HW2, 128, 128), f)
        blk[:, 0:64, 0:64] = W2
        blk[:, 64:128, 64:128] = W2
        wpd.append(np.ascontiguousarray(
            blk.transpose(1, 0, 2)).reshape(128, HW2 * 128))
    con["Wpd"] = np.stack(wpd).astype(ml_dtypes.bfloat16)
    con["bp"] = np.concatenate(
        [np.concatenate([np.asarray(b, f)] * 2) for b in (bpq, bpk, bpv)]
    )[None, :].astype(ml_dtypes.bfloat16)
    gs = [np.asarray(g_q, f), np.asarray(g_k, f) * SCALE, np.asarray(g_v, f)]
    bs = [np.asarray(be_q, f), np.asarray(be_k, f) * SCALE, np.asarray(be_v, f)]
    con["lng"] = np.stack([np.broadcast_to(g, (128, C)) for g in gs]).astype(f)
    con["lnb"] = np.stack([np.broadcast_to(b, (128, C)) for b in bs]).astype(f)
    rT = np.ascontiguousarray(np.asarray(rel_pos_t, f).T)      # [C, 127]
    con["relT"] = np.concatenate([rT, rT], axis=0).astype(ml_dtypes.bfloat16)
    con["idbf"] = np.eye(128).astype(ml_dtypes.bfloat16)
    con["idf"] = np.eye(128).astype(f)
    con["Wpr"] = np.ascontiguousarray(
        np.asarray(W_proj, f).reshape(KT, 128, D)).astype(ml_dtypes.bfloat16)
    con["bpr"] = np.asarray(b_proj, f)[None, :].astype(ml_dtypes.bfloat16)
    con["onesv"] = np.ones((1, BT), ml_dtypes.bfloat16)
    return con


def kernel(x, W_qkv, Wpq, bpq, Wpk, bpk, Wpv, bpv,
           g_q, be_q, g_k, be_k, g_v, be_v, rel_pos_t, W_proj, b_proj):
    x = np.asarray(x, np.float32)
    if "nc" not in _cache:
        _cache["nc"] = _build_nc()
    nc = _cache["nc"]

    con = _prep_consts(W_qkv, Wpq, bpq, Wpk, bpk, Wpv, bpv,
                       g_q, be_q, g_k, be_k, g_v, be_v,
                       rel_pos_t, W_proj, b_proj)
    in_maps = []
    for i in range(NCORES):
        xs = x[i * BLOC:(i + 1) * BLOC].reshape(ROWS, D)
        xsT = np.ascontiguousarray(xs.T).reshape(KT, 128, ROWS).astype(
            ml_dtypes.bfloat16)
        m = dict(con)
        m["xT"] = xsT
        in_maps.append(m)

    res = run_bass_kernel_spmd(nc, in_maps, core_ids=list(range(NCORES)))
    _cache["last_exec_ns"] = res.exec_time_ns
    out = np.concatenate(
        [np.asarray(res.results[i]["out"], np.float32).reshape(BLOC, T, D)
         for i in range(NCORES)], axis=0)
    return out
